# revision 9
# baseline (speedup 1.0000x reference)
"""Causal self-attention (GQA + RoPE + QK-RMSNorm) on 8 trn2 NeuronCores.

Reference (B=2, T=2048, C=2048, 16 q-heads / 4 kv-heads, head_dim 128):
    q = rms_norm(rope(x @ Wq)) / sqrt(128); k = rms_norm(rope(x @ Wk))
    att = softmax_causal(q k^T / sqrt(128)); y = (att @ v) @ Wp

Sharding: core = 4*b + g (b = batch, g = head-group).  Each core
projects+attends q-heads 4g..4g+3 (kv-head g) over the full causal
sequence of its batch, computes the partial output projection
y_local @ Wp[local-head rows, :] over all T, and ReduceScatter(add)
ops per (512-t-chunk, 512-cout-block) sum the 4 partials and shard
cout 128-wide; the host places disjoint shards.

Perf structure:
 - Q/K projections in fp8 (e4m3) DoubleRow (2x PE rate); rms-norm
   cancels any weight scale so Wq/Wk are pre-scaled 64x into fp8
   range.  V and Wp stay bf16 for accuracy.
 - rms reciprocals via DVE Newton rsqrt (bit-trick seed): the scalar
   engine only ever runs Exp and Copy -> zero act-table reloads.
 - Attention S^T layout: exp(scale*s) with per-partition 1/rms_k
   scale; causal handled block-wise, diagonal blocks column-trimmed,
   one [128,512] (j>=p) mask.
 - tqc-outer emission pipelines proj -> attention -> out-proj -> RS
   per 512-t chunk; RS sub-split per cout block for overlap.
"""

import math

import ml_dtypes
import numpy as np

B, T, C = 2, 2048, 2048
NH, NKV, HD = 16, 4, 128
G = 4  # q-heads per core
EPS = 1e-6
NCB = C // 128  # 16 contraction blocks
NPAIR = NCB // 2  # 8 fp8 DoubleRow pairs
NTCH = T // 512  # 4 t-chunks
NTKB = T // 128  # 16 key blocks
MAGIC = 0x5F3759DF

_CACHE = {}


def _build():
    import concourse.mybir as mybir
    import concourse.tile as tile
    from concourse import bacc
    from concourse.masks import make_identity
    from contextlib import ExitStack

    F32 = mybir.dt.float32
    I32 = mybir.dt.int32
    BF16 = mybir.dt.bfloat16
    F8 = mybir.dt.float8e4
    AF = mybir.ActivationFunctionType
    ALU = mybir.AluOpType
    DR = mybir.MatmulPerfMode.DoubleRow

    nc = bacc.Bacc(None, target_bir_lowering=False, num_devices=8)

    x8 = nc.dram_tensor("x8", [128, NPAIR, 2, T], F8, kind="ExternalInput")
    xb = nc.dram_tensor("xb", [128, NCB, T], BF16, kind="ExternalInput")
    wq8 = nc.dram_tensor("wq8", [128, NPAIR, 2, G * HD], F8, kind="ExternalInput")
    wk8 = nc.dram_tensor("wk8", [128, NPAIR, 2, HD], F8, kind="ExternalInput")
    wv = nc.dram_tensor("wv", [128, NCB, HD], BF16, kind="ExternalInput")
    wp = nc.dram_tensor("wp", [128, G, C], BF16, kind="ExternalInput")
    cos2 = nc.dram_tensor("cos2", [128, T], BF16, kind="ExternalInput")
    sin2 = nc.dram_tensor("sin2", [128, T], BF16, kind="ExternalInput")
    maskin = nc.dram_tensor("maskin", [128, 512], BF16, kind="ExternalInput")
    outT = nc.dram_tensor("outT", [NTCH, 4, 128, 512], BF16, kind="ExternalOutput")

    groups = [[0, 1, 2, 3], [4, 5, 6, 7]]

    with tile.TileContext(nc) as tc:
        with ExitStack() as es:
            dram = es.enter_context(tc.tile_pool(name="dram", bufs=4, space="DRAM"))

            consts = es.enter_context(tc.tile_pool(name="consts", bufs=1))
            ones_bf = consts.tile([128, 1], BF16)
            nc.vector.memset(ones_bf[:], 1.0)
            ident_bf = consts.tile([128, 128], BF16)
            make_identity(nc, ident_bf[:])
            cos_sb = consts.tile([128, T], BF16)
            sin_sb = consts.tile([128, T], BF16)
            mask_sb = consts.tile([128, 512], BF16)

            wpool = es.enter_context(tc.tile_pool(name="w", bufs=1))
            wq_sb = wpool.tile([128, NPAIR, 2, G * HD], F8)
            wk_sb = wpool.tile([128, NPAIR, 2, HD], F8)
            wv_sb = wpool.tile([128, NCB, HD], BF16)
            wp_sb = wpool.tile([128, G, C], BF16)
            nc.sync.dma_start(out=wk_sb[:], in_=wk8[:])
            nc.sync.dma_start(out=wq_sb[:], in_=wq8[:])
            nc.sync.dma_start(out=wv_sb[:], in_=wv[:])
            nc.sync.dma_start(out=cos_sb[:], in_=cos2[:])
            nc.sync.dma_start(out=sin_sb[:], in_=sin2[:])
            nc.sync.dma_start(out=mask_sb[:], in_=maskin[:])
            # wp is not needed until the first out-proj; keep it off the
            # critical sync-queue prefix
            nc.gpsimd.dma_start(out=wp_sb[:], in_=wp[:])

            acts = es.enter_context(tc.tile_pool(name="acts", bufs=1))
            qT_sb = acts.tile([128, G, T], BF16)
            kT_sb = acts.tile([128, T], BF16)
            v_sb = acts.tile([128, NTKB, HD], BF16)
            yT_sb = acts.tile([128, G, T], BF16)
            rk_col = acts.tile([128, NTKB], F32)

            xs = es.enter_context(tc.tile_pool(name="xs", bufs=2))
            tmp = es.enter_context(tc.tile_pool(name="tmp", bufs=3))
            qro = es.enter_context(tc.tile_pool(name="qro", bufs=6))
            sml = es.enter_context(tc.tile_pool(name="sml", bufs=3))
            pts = es.enter_context(tc.tile_pool(name="pts", bufs=5))
            pos = es.enter_context(tc.tile_pool(name="pos", bufs=3))
            # PSUM: role-separated pools; 8 banks total
            pp = es.enter_context(tc.tile_pool(name="pp", bufs=2, space="PSUM"))
            pss = es.enter_context(tc.tile_pool(name="pss", bufs=2, space="PSUM"))
            psy = es.enter_context(tc.tile_pool(name="psy", bufs=2, space="PSUM"))
            psr = es.enter_context(tc.tile_pool(name="psr", bufs=1, space="PSUM"))
            psm = es.enter_context(tc.tile_pool(name="psm", bufs=1, space="PSUM"))

            def rope(dst, src_psum, tcs):
                """dst = src*cos + rotate_half(src)*sin over t-cols tcs."""
                rot = tmp.tile([128, 512], BF16, tag="rot")
                nc.scalar.copy(out=rot[0:64, :], in_=src_psum[64:128, :])
                nc.scalar.copy(out=rot[64:128, :], in_=src_psum[0:64, :])
                qr = tmp.tile([128, 512], F32, tag="qr")
                nc.vector.tensor_mul(qr[:], src_psum[:], cos_sb[:, tcs])
                nc.vector.tensor_mul(rot[:], rot[:], sin_sb[:, tcs])
                nc.vector.tensor_add(dst, qr[:], rot[:])

            def rsqrt(dst, src_psum, n, scale, bias, tag):
                """dst = 1/sqrt(src*scale + bias), Newton x2 on DVE."""
                x = sml.tile([128, n], F32, tag=tag + "x")
                nc.vector.tensor_scalar(
                    x[:], src_psum[:], scale, bias, ALU.mult, ALU.add
                )
                y = sml.tile([128, n], F32, tag=tag + "y")
                yi = y[:].bitcast(I32)
                nc.vector.tensor_scalar(
                    yi, x[:].bitcast(I32), 1, None, ALU.logical_shift_right
                )
                nc.vector.tensor_scalar(yi, yi, -1, MAGIC, ALU.mult, ALU.add)
                t1 = sml.tile([128, n], F32, tag=tag + "t")
                for _ in range(2):
                    nc.vector.tensor_mul(t1[:], y[:], y[:])
                    nc.vector.tensor_mul(t1[:], t1[:], x[:])
                    nc.vector.tensor_scalar(
                        t1[:], t1[:], -0.5, 1.5, ALU.mult, ALU.add
                    )
                    nc.vector.tensor_mul(y[:], y[:], t1[:])
                nc.vector.tensor_copy(out=dst, in_=y[:])

            for tch in range(NTCH):
                tcs = slice(512 * tch, 512 * tch + 512)

                x8_t = xs.tile([128, NPAIR, 2, 512], F8, tag="x8")
                nc.sync.dma_start(out=x8_t[:], in_=x8[:, :, :, tcs])
                xb_t = xs.tile([128, NCB, 512], BF16, tag="xb")
                nc.sync.dma_start(out=xb_t[:], in_=xb[:, :, tcs])

                # ---- K projection (fp8 DoubleRow) + rope + 1/rms ----
                ps_k = pp.tile([128, 512], F32, tag="pp")
                for j in range(NPAIR):
                    nc.tensor.matmul(
                        ps_k[:], wk_sb[:, j], x8_t[:, j],
                        start=(j == 0), stop=(j == NPAIR - 1), perf_mode=DR,
                    )
                rope(kT_sb[:, tcs], ps_k[:], tcs)
                ksq = sml.tile([128, 512], BF16, tag="ksq")
                nc.vector.tensor_mul(ksq[:], kT_sb[:, tcs], kT_sb[:, tcs])
                ps_kc = psm.tile([128, 4], F32, tag="psm")
                for jj in range(4):
                    nc.tensor.matmul(
                        ps_kc[:, jj : jj + 1],
                        ksq[:, 128 * jj : 128 * jj + 128],
                        ones_bf[:],
                        start=True, stop=True,
                    )
                # rk = 1/rms_k = 1/sqrt(ss/HD + eps)
                rsqrt(
                    rk_col[:, 4 * tch : 4 * tch + 4], ps_kc[:], 4,
                    1.0 / HD, EPS, "rk",
                )

                # ---- V projection (bf16) + PE transpose into v_sb ----
                ps_v = pp.tile([128, 512], F32, tag="pp")
                for cb in range(NCB):
                    nc.tensor.matmul(
                        ps_v[:], wv_sb[:, cb], xb_t[:, cb],
                        start=(cb == 0), stop=(cb == NCB - 1),
                    )
                vbf = sml.tile([128, 512], BF16, tag="vbf")
                nc.scalar.copy(out=vbf[:], in_=ps_v[:])
                for tt in range(4):
                    ps_tr = psm.tile([128, 128], BF16, tag="psm")
                    nc.tensor.transpose(
                        ps_tr[:], vbf[:, 128 * tt : 128 * tt + 128], ident_bf[:]
                    )
                    nc.vector.tensor_copy(out=v_sb[:, 4 * tch + tt, :], in_=ps_tr[:])

                # ---- Q projections (fp8 DoubleRow) + rope + 1/(HD*rms) ----
                # per-query norms in column layout: ps_qc col 4h+jj = head h,
                # 128-t-block jj
                qropes = []
                ps_qc = psm.tile([128, 16], F32, tag="psm")
                for h in range(G):
                    ps_q = pp.tile([128, 512], F32, tag="pp")
                    for j in range(NPAIR):
                        nc.tensor.matmul(
                            ps_q[:],
                            wq_sb[:, j, :, 128 * h : 128 * h + 128],
                            x8_t[:, j],
                            start=(j == 0), stop=(j == NPAIR - 1), perf_mode=DR,
                        )
                    qrope = qro.tile([128, 512], BF16, tag="qro")
                    rope(qrope[:], ps_q[:], tcs)
                    qropes.append(qrope)
                    sq = sml.tile([128, 512], BF16, tag="sq")
                    nc.vector.tensor_mul(sq[:], qrope[:], qrope[:])
                    for jj in range(4):
                        nc.tensor.matmul(
                            ps_qc[:, 4 * h + jj : 4 * h + jj + 1],
                            sq[:, 128 * jj : 128 * jj + 128],
                            ones_bf[:],
                            start=True, stop=True,
                        )
                # rq = 1/(HD*rms_q) = 1/sqrt(HD*ss + HD^2*eps)
                rqc = sml.tile([128, 16], BF16, tag="rqc")
                rsqrt(rqc[:], ps_qc[:], 16, float(HD), float(HD * HD) * EPS, "rq")
                # transpose norm columns to rows, bounce via DRAM to get all
                # 16 rows onto partition 0 (direct SBUF flatten fails load)
                ps_rq = psm.tile([16, 128], BF16, tag="psm")
                nc.tensor.transpose(ps_rq[:], rqc[:], ident_bf[:])
                rq16 = sml.tile([16, 128], BF16, tag="rqs")
                nc.vector.tensor_copy(out=rq16[:], in_=ps_rq[:])
                drq = dram.tile([16, 128], BF16, tag="drq")
                nc.sync.dma_start(out=drq[:], in_=rq16[:])
                rq_row = sml.tile([1, 2048], BF16, tag="rqrow")
                nc.sync.dma_start(
                    out=rq_row[:], in_=drq[:].rearrange("p m -> (p m)")
                )
                for h in range(G):
                    bcq = sml.tile([128, 512], BF16, tag="bcq")
                    nc.gpsimd.partition_broadcast(
                        bcq[:], rq_row[0:1, 512 * h : 512 * h + 512]
                    )
                    nc.vector.tensor_mul(qT_sb[:, h, tcs], qropes[h][:], bcq[:])

            # ---- phase 2+4, tqc outer: attention, out-proj, ReduceScatter ----
            for tqc in range(NTCH):
                tqs = slice(512 * tqc, 512 * tqc + 512)
                for h in range(G):
                    ps_y = psy.tile([128, 512], F32, tag="psy")
                    ps_rs = psr.tile([1, 512], F32, tag="psr")
                    for tkb in range(4 * tqc):  # full blocks
                        ps_s = pss.tile([128, 512], F32, tag="pss")
                        nc.tensor.matmul(
                            ps_s[:],
                            kT_sb[:, 128 * tkb : 128 * tkb + 128],
                            qT_sb[:, h, tqs],
                            start=True, stop=True,
                        )
                        pT = pts.tile([128, 512], BF16, tag="pt")
                        nc.scalar.activation(
                            out=pT[:], in_=ps_s[:], func=AF.Exp,
                            scale=rk_col[:, tkb : tkb + 1],
                        )
                        nc.tensor.matmul(
                            ps_rs[:], ones_bf[:], pT[:],
                            start=(tkb == 0), stop=False,
                        )
                        nc.tensor.matmul(
                            ps_y[:], v_sb[:, tkb, :], pT[:],
                            start=(tkb == 0), stop=False,
                        )
                    for dd in range(4):  # diagonal blocks, trimmed
                        tkb = 4 * tqc + dd
                        w = 512 - 128 * dd
                        qs = slice(512 * tqc + 128 * dd, 512 * tqc + 512)
                        ps_s = pss.tile([128, 512], F32, tag="pss")
                        nc.tensor.matmul(
                            ps_s[:, 0:w],
                            kT_sb[:, 128 * tkb : 128 * tkb + 128],
                            qT_sb[:, h, qs],
                            start=True, stop=True,
                        )
                        pT = pts.tile([128, 512], BF16, tag="pt")
                        nc.scalar.activation(
                            out=pT[:, 0:w], in_=ps_s[:, 0:w], func=AF.Exp,
                            scale=rk_col[:, tkb : tkb + 1],
                        )
                        nc.vector.tensor_mul(
                            pT[:, 0:w], pT[:, 0:w], mask_sb[:, 0:w]
                        )
                        nc.tensor.matmul(
                            ps_rs[:, 128 * dd : 512], ones_bf[:], pT[:, 0:w],
                            start=(tkb == 0), stop=(dd == 3),
                        )
                        nc.tensor.matmul(
                            ps_y[:, 128 * dd : 512], v_sb[:, tkb, :], pT[:, 0:w],
                            start=(tkb == 0), stop=(dd == 3),
                        )
                    rrow = sml.tile([1, 512], F32, tag="rrow")
                    nc.vector.reciprocal_approx_fast(out=rrow[:], in_=ps_rs[:])
                    bc = sml.tile([128, 512], F32, tag="bc")
                    nc.gpsimd.partition_broadcast(bc[:], rrow[:])
                    nc.vector.tensor_mul(yT_sb[:, h, tqs], ps_y[:], bc[:])

                # ---- partial out-proj for this t-chunk; RS per cout block ----
                partial = dram.tile([C, 512], BF16, tag="partial")
                for kk in range(4):  # 512-wide cout block -> one sub-RS
                    for co in range(4):
                        cob = 4 * kk + co
                        ps_o = pp.tile([128, 512], F32, tag="pp")
                        for h2 in range(G):
                            nc.tensor.matmul(
                                ps_o[:],
                                wp_sb[:, h2, 128 * cob : 128 * cob + 128],
                                yT_sb[:, h2, tqs],
                                start=(h2 == 0), stop=(h2 == G - 1),
                            )
                        po = pos.tile([128, 512], BF16, tag="po")
                        nc.vector.tensor_copy(out=po[:], in_=ps_o[:])
                        nc.sync.dma_start(
                            out=partial[128 * cob : 128 * cob + 128, :], in_=po[:]
                        )
                    rs_out = dram.tile([128, 512], BF16, tag="rsout")
                    nc.gpsimd.collective_compute(
                        "ReduceScatter",
                        mybir.AluOpType.add,
                        replica_groups=groups,
                        ins=[partial[512 * kk : 512 * kk + 512, :]],
                        outs=[rs_out[:]],
                    )
                    nc.sync.dma_start(out=outT[tqc, kk], in_=rs_out[:])

    nc.compile()
    return nc


def _get_nc():
    if "nc" not in _CACHE:
        _CACHE["nc"] = _build()
    return _CACHE["nc"]


def _prep_core_inputs(x, cos, sin, Wq, Wk, Wv, Wp):
    f32 = np.float32
    bf16 = ml_dtypes.bfloat16
    f8 = ml_dtypes.float8_e4m3
    cosT = np.asarray(cos, dtype=f32).T  # [64, T]
    sinT = np.asarray(sin, dtype=f32).T
    cos2 = np.ascontiguousarray(np.vstack([cosT, cosT])).astype(bf16)
    sin2 = np.ascontiguousarray(np.vstack([-sinT, sinT])).astype(bf16)
    p = np.arange(128)[:, None]
    j = np.arange(512)[None, :]
    maskin = (j >= p).astype(bf16)

    in_maps = []
    for core in range(8):
        b, g = core // 4, core % 4
        xT = np.ascontiguousarray(np.asarray(x)[b].T).astype(f32)  # [C, T]
        x8 = np.ascontiguousarray(
            xT.reshape(NPAIR, 2, 128, T).transpose(2, 0, 1, 3)
        ).astype(f8)
        xbm = np.ascontiguousarray(
            xT.reshape(NCB, 128, T).transpose(1, 0, 2)
        ).astype(bf16)
        wq8 = np.ascontiguousarray(
            (np.asarray(Wq)[:, 512 * g : 512 * g + 512] * 64.0)
            .reshape(NPAIR, 2, 128, 512)
            .transpose(2, 0, 1, 3)
        ).astype(f8)
        wk8 = np.ascontiguousarray(
            (np.asarray(Wk)[:, 128 * g : 128 * g + 128] * 64.0)
            .reshape(NPAIR, 2, 128, 128)
            .transpose(2, 0, 1, 3)
        ).astype(f8)
        wvm = np.ascontiguousarray(
            np.asarray(Wv)[:, 128 * g : 128 * g + 128]
            .reshape(NCB, 128, 128)
            .transpose(1, 0, 2)
        ).astype(bf16)
        wpm = np.ascontiguousarray(
            np.asarray(Wp)[512 * g : 512 * g + 512, :]
            .reshape(G, 128, C)
            .transpose(1, 0, 2)
        ).astype(bf16)
        in_maps.append(
            {
                "x8": x8, "xb": xbm, "wq8": wq8, "wk8": wk8, "wv": wvm,
                "wp": wpm, "cos2": cos2, "sin2": sin2, "maskin": maskin,
            }
        )
    return in_maps


def kernel(x, cos, sin, Wq, Wk, Wv, Wp):
    from concourse.bass_utils import run_bass_kernel_spmd

    in_maps = _prep_core_inputs(x, cos, sin, Wq, Wk, Wv, Wp)
    nc = _get_nc()
    res = run_bass_kernel_spmd(nc, in_maps, core_ids=list(range(8)), trace=False)

    out = np.empty((B, T, C), dtype=np.float32)
    for core in range(8):
        b, g = core // 4, core % 4
        o = np.asarray(res.results[core]["outT"], dtype=np.float32)  # [4,4,128,512]
        for tch in range(NTCH):
            for kk in range(4):
                cstart = 512 * kk + 128 * g
                out[b, 512 * tch : 512 * tch + 512, cstart : cstart + 128] = o[
                    tch, kk
                ].T
    return out
